# revision 39
# baseline (speedup 1.0000x reference)
"""DeepSeek-MoE layer on 8 Trainium2 NeuronCores (expert-parallel).

Strategy (v2)
-------------
- Routing (affinity matmul + biased top-8 + sigmoid weights) is computed
  on-device, token-sharded in exact fp32; the combine-weight matrix
  cw [2048, 64] is AllGathered.
- Each core owns 8 experts. Per-expert gather lists are built on-device:
  mask -> positions via a triangular-matmul cumsum -> slot->token map via
  per-expert one-hot matmuls (hi/lo token-id split keeps everything exact
  in bf16). Combine weights ride along the x-gather: the device writes
  bf16 (hi, lo) pairs of cw into spare columns of the gather source, so
  no separate weight gather is needed.
- Expert FFN in fp8e4m3 with DoubleRow perf mode (256-deep contraction
  per matmul instruction, half the instruction count of bf16).
- Combine: per-slot outputs are scaled by their combine weight, written
  bf16, and scatter-added (indirect DMA, cce add) into a bf16 token
  accumulator; a bf16 ReduceScatter leaves each core its 256-token shard.
  Scatters serialize per-expert (slots within an expert are distinct
  tokens, so an expert's 3 chunk scatters run concurrently).
- The shared expert runs in bf16 on the token shard, overlapping the
  ReduceScatter; host concatenates the 8 shards.
"""
import sys

sys.path.insert(0, "/opt/trn_rl_repo")

import os

import numpy as np

from concourse import bass, bacc, mybir
import concourse.tile as tile
from concourse.tile import add_dep_helper

# problem shapes (hardcoded per contract)
B, S, D, F, E, K = 2, 1024, 1024, 512, 64, 8
T = B * S                # 2048 tokens
N_CORES = 8
EL = E // N_CORES        # 8 local experts per core
C = 384                  # capacity per expert (max observed load 305)
CCH = C // 128           # 3 slot chunks per expert
NSL = EL * C             # 3072 local slots
NCH = NSL // 128         # 24 slot chunks per core
NT = T // 128            # 16 token tiles
TS = T // N_CORES        # 256 tokens per core shard
SENT = -1e30
NO_AG = os.environ.get("MOE_NO_AG") == "1"
NO_RS = os.environ.get("MOE_NO_RS") == "1"
OOB = 2048  # one past the last valid token index; > bounds_check -> skipped
AUG = 2 * EL             # 16 extra bf16 columns carrying cw (hi, lo) pairs
DA = D + AUG

FP = mybir.dt.float32
FR = mybir.dt.float32r
FH = mybir.dt.float16
BF = mybir.dt.bfloat16
F8 = mybir.dt.float8e4
I32 = mybir.dt.int32
DR = mybir.MatmulPerfMode.DoubleRow


def _host_constants():
    ident = np.eye(128, dtype=np.float32)
    # Ucomb[:, :128] strict upper triangular ones (exclusive within-chunk
    # cumsum); col 128 = ones (chunk totals); cols 129..135 zero pad.
    ucomb = np.zeros((128, 136), dtype=np.float32)
    ucomb[:, :128] = np.triu(np.ones((128, 128), dtype=np.float32), k=1)
    ucomb[:, 128] = 1.0
    tri16 = np.triu(np.ones((16, 16), dtype=np.float32), k=1)  # strict upper
    iota384 = np.tile(np.arange(C, dtype=np.float32), (128, 1))  # [128, 384]
    # tok3: per token tile i: [128*i, partition index, 1]
    tok3 = np.zeros((128, 3 * NT), dtype=np.float32)
    for t in range(NT):
        tok3[:, 3 * t] = 128.0 * t
        tok3[:, 3 * t + 1] = np.arange(128)
        tok3[:, 3 * t + 2] = 1.0
    return ident, ucomb, tri16, iota384, tok3


def build_kernel():
    nc = bacc.Bacc(target_bir_lowering=False)

    # ---------------- I/O ----------------
    # exact-fp32 routing inputs
    xTs = nc.dram_tensor("xTs", [D, TS], FP, kind="ExternalInput")        # per-core x-shard, transposed
    cenT = nc.dram_tensor("cenT", [D, E], FP, kind="ExternalInput")       # centroids^T (replicated)
    bias128 = nc.dram_tensor("bias128", [128, E], FP, kind="ExternalInput")
    # gather source: bf16 x rows + 16 spare cols (device writes cw pairs)
    x_aug = nc.dram_tensor("x_aug", [T, DA], BF, kind="ExternalInput")
    # fp8 expert weights in DoubleRow pair layout
    wu8 = nc.dram_tensor("wu8", [EL, D // 256, 128, 2, F], F8, kind="ExternalInput")
    wd8 = nc.dram_tensor("wd8", [EL, F // 256, 128, 2, D], F8, kind="ExternalInput")
    # shared expert weights (bf16)
    wsu = nc.dram_tensor("wsu", [D, F], BF, kind="ExternalInput")
    wsd = nc.dram_tensor("wsd", [F, D], BF, kind="ExternalInput")
    sel64 = nc.dram_tensor("sel64", [E, EL], FR, kind="ExternalInput")

    out_shard = nc.dram_tensor("out_shard", [TS, D], FP, kind="ExternalOutput")

    # internal DRAM
    cw_sh = nc.dram_tensor("cw_sh", [TS, E], FP)                  # this core's cw shard
    cw_all = nc.dram_tensor("cw_all", [T, E], FP, addr_space="Shared")  # AllGather output
    acc = nc.dram_tensor("acc_dram", [T, D], BF)                  # scatter-add target / RS input
    rs_out = nc.dram_tensor("rs_out", [TS, D], BF)                # RS output shard

    # constants passed as inputs (inline_tensor is untested under the pjrt path)
    ident_dr = nc.dram_tensor("ident_c", [128, 128], FP, kind="ExternalInput")
    ucomb_dr = nc.dram_tensor("ucomb_c", [128, 136], BF, kind="ExternalInput")
    tri16_dr = nc.dram_tensor("tri16_c", [16, 16], FH, kind="ExternalInput")
    iota_dr = nc.dram_tensor("iota_c", [128, C], FH, kind="ExternalInput")
    tok3_dr = nc.dram_tensor("tok3_c", [128, 3 * NT], BF, kind="ExternalInput")

    with (
        tile.TileContext(nc) as tc,
        tc.tile_pool(name="const", bufs=1) as cpool,
        tc.tile_pool(name="route", bufs=2) as rpool,
        tc.tile_pool(name="gbuild", bufs=2) as gpool,
        tc.tile_pool(name="persist", bufs=1) as ppool,
        tc.tile_pool(name="wpool", bufs=2) as wpool,
        tc.tile_pool(name="fpool", bufs=2) as fpool,
        tc.tile_pool(name="psA", bufs=1, space="PSUM") as psA,
        tc.tile_pool(name="psG", bufs=1, space="PSUM") as psG,
    ):
        # ---------------- constants to SBUF ----------------
        ident = cpool.tile([128, 128], FP)
        nc.sync.dma_start(out=ident[:], in_=ident_dr[:, :])
        ucomb = cpool.tile([128, 136], BF)
        nc.sync.dma_start(out=ucomb[:], in_=ucomb_dr[:, :])
        tri16 = cpool.tile([16, 16], FH)
        nc.sync.dma_start(out=tri16[:], in_=tri16_dr[:, :])
        iota384 = cpool.tile([128, C], FH)
        nc.sync.dma_start(out=iota384[:], in_=iota_dr[:, :])
        tok3 = cpool.tile([128, 3 * NT], BF)
        nc.sync.dma_start(out=tok3[:], in_=tok3_dr[:, :])
        bias_t = cpool.tile([128, E], FP)
        nc.sync.dma_start(out=bias_t[:], in_=bias128[:, :])
        sel_t = cpool.tile([E, EL], FR)
        nc.sync.dma_start(out=sel_t[:], in_=sel64[:, :])
        identb = cpool.tile([128, 128], BF)
        nc.vector.tensor_copy(out=identb[:], in_=ident[:])

        # warmup transpose so PE observes ident's clock early
        warm_ps = psA.tile([128, 128], FP, space="PSUM", tag="small", bufs=2)
        nc.tensor.transpose(out=warm_ps[:], in_=ident[:], identity=ident[:])

        # ---------------- phase R: routing on this core's 256-token shard ----------------
        xts_sb = []   # [128, TS] fp32 tiles of xT_shard (d-chunks)
        for kk in range(D // 128):
            xt = rpool.tile([128, TS], FP, tag="xts", bufs=8)
            nc.sync.dma_start(out=xt[:], in_=xTs[kk * 128:(kk + 1) * 128, :])
            xts_sb.append(xt)
        cen_sb = []
        for kk in range(D // 128):
            ct = rpool.tile([128, E], FP, tag="cen", bufs=8)
            nc.sync.dma_start(out=ct[:], in_=cenT[kk * 128:(kk + 1) * 128, :])
            cen_sb.append(ct)

        for tt in range(TS // 128):  # 2 tiles
            aff_ps = psA.tile([128, E], FP, space="PSUM", tag="small", bufs=2)
            for kk in range(D // 128):
                nc.tensor.matmul(
                    out=aff_ps[:],
                    lhsT=xts_sb[kk][:, tt * 128:(tt + 1) * 128],
                    rhs=cen_sb[kk][:],
                    start=(kk == 0),
                    stop=(kk == D // 128 - 1),
                )
            aff = rpool.tile([128, E], FP, tag="aff")
            nc.vector.tensor_copy(out=aff[:], in_=aff_ps[:])
            biased = rpool.tile([128, E], FP, tag="biased")
            nc.vector.tensor_add(out=biased[:], in0=aff[:], in1=bias_t[:])
            top8 = rpool.tile([128, 8], FP, tag="top8")
            nc.vector.max(out=top8[:], in_=biased[:])
            masked = rpool.tile([128, E], FP, tag="masked")
            nc.vector.match_replace(
                out=masked[:], in_to_replace=top8[:], in_values=biased[:],
                imm_value=SENT,
            )
            msk = rpool.tile([128, E], FP, tag="msk")
            nc.vector.tensor_scalar(
                out=msk[:], in0=masked[:], scalar1=SENT, scalar2=None,
                op0=mybir.AluOpType.is_equal,
            )
            sig = rpool.tile([128, E], FP, tag="sig")
            nc.scalar.activation(out=sig[:], in_=aff[:],
                                 func=mybir.ActivationFunctionType.Sigmoid)
            wdense = rpool.tile([128, E], FP, tag="wdense")
            nc.vector.tensor_mul(out=wdense[:], in0=sig[:], in1=msk[:])
            tsum = rpool.tile([128, 32], FP, tag="tsum")
            nc.vector.tensor_add(out=tsum[:], in0=wdense[:, 0:32], in1=wdense[:, 32:64])
            for w_ in (16, 8, 4, 2, 1):
                nc.vector.tensor_add(out=tsum[:, 0:w_], in0=tsum[:, 0:w_],
                                     in1=tsum[:, w_:2 * w_])
            denom = rpool.tile([128, 1], FP, tag="denom")
            nc.vector.tensor_scalar_add(denom[:], tsum[:, 0:1], 1e-8)
            recip = rpool.tile([128, 1], FP, tag="recip")
            nc.vector.reciprocal(out=recip[:], in_=denom[:])
            cw_t = rpool.tile([128, E], FP, tag="cwt")
            nc.vector.tensor_scalar_mul(cw_t[:], wdense[:], recip[:, :1])
            nc.sync.dma_start(out=cw_sh[tt * 128:(tt + 1) * 128, :], in_=cw_t[:])

        if NO_AG:
            for rrep in range(N_CORES):
                ag = nc.sync.dma_start(out=cw_all[rrep * TS:(rrep + 1) * TS, :],
                                       in_=cw_sh[:, :])
        else:
            ag = nc.gpsimd.collective_compute(
                "AllGather",
                mybir.AluOpType.bypass,
                ins=[cw_sh.ap().opt()],
                outs=[cw_all.ap().opt()],
                replica_groups=[list(range(N_CORES))],
            )

        # zero tile + ACC memset (after routing issue; overlaps AllGather)
        zero_t = cpool.tile([128, D], BF)
        nc.vector.memset(zero_t[:], 0.0)
        memset_insts = []
        for i in range(NT):
            mi = nc.sync.dma_start(out=acc[i * 128:(i + 1) * 128, :], in_=zero_t[:])
            memset_insts.append(mi.ins)

        # ---------------- phase P: positions + per-token combine weights ----------------
        # cw_all -> cwl [128, EL] per tile (local-expert columns) via
        # batched transpose + selection matmul (per-core column pick).
        cwa_tiles = []
        for i in range(NT):
            cwa = gpool.tile([128, E], FP, tag="cwa", bufs=4)
            ld = nc.sync.dma_start(out=cwa[:], in_=cw_all[i * 128:(i + 1) * 128, :])
            add_dep_helper(ld.ins, ag.ins)
            cwa_tiles.append(cwa)
        cwlT_sb = []  # 4 tiles [8, 512] fp32 (cw^T of local experts)
        for b4 in range(NT // 4):
            caT_ps = psA.tile([E, 512], FP, space="PSUM", tag="small", bufs=2)
            for j in range(4):
                nc.tensor.transpose(
                    out=caT_ps[:, j * 128:(j + 1) * 128],
                    in_=cwa_tiles[4 * b4 + j][:],
                    identity=ident[:],
                )
            caT = gpool.tile([E, 512], FR, tag="caT", bufs=2)
            nc.vector.tensor_copy(out=caT[:], in_=caT_ps[:])
            clT_ps = psA.tile([EL, 512], FP, space="PSUM", tag="small", bufs=2)
            nc.tensor.matmul(out=clT_ps[:], lhsT=sel_t[:], rhs=caT[:],
                             start=True, stop=True)
            clT = gpool.tile([EL, 512], FP, tag="clT", bufs=4)
            nc.vector.tensor_copy(out=clT[:], in_=clT_ps[:])
            cwlT_sb.append(clT)

        p_t = ppool.tile([8, T], FP, tag="p_t")          # P^T: per local expert, exclusive counts
        totals = ppool.tile([8, NT], FP, tag="totals")   # per-chunk totals
        cwl_tiles = []
        for i in range(NT):
            cwl_ps = psA.tile([128, EL], FP, space="PSUM", tag="small", bufs=2)
            nc.tensor.transpose(
                out=cwl_ps[:],
                in_=cwlT_sb[i // 4][:, (i % 4) * 128:(i % 4 + 1) * 128],
                identity=ident[:EL, :EL],
            )
            cwl = ppool.tile([128, EL], FP, tag="cwl", bufs=16)
            nc.vector.tensor_copy(out=cwl[:], in_=cwl_ps[:])
            cwl_tiles.append(cwl)
            mlb = ppool.tile([128, EL], BF, tag="mlb", bufs=2)
            nc.vector.tensor_scalar(
                out=mlb[:], in0=cwl[:], scalar1=0.0, scalar2=None,
                op0=mybir.AluOpType.is_gt,
            )
            cum_ps = psA.tile([8, 136], FP, space="PSUM", tag="small", bufs=2)
            nc.tensor.matmul(out=cum_ps[:], lhsT=mlb[:], rhs=ucomb[:],
                             start=True, stop=True)
            nc.vector.tensor_copy(out=p_t[:, i * 128:(i + 1) * 128], in_=cum_ps[:, :128])
            nc.vector.tensor_copy(out=totals[:, i:i + 1], in_=cum_ps[:, 128:129])

        # chunk-prefix: totalsT = totals^T [16, 8] -> prefix [8, 16]
        totT_ps = psA.tile([16, 8], FP, space="PSUM", tag="small", bufs=2)
        nc.tensor.transpose(out=totT_ps[:], in_=totals[:], identity=ident[:8, :8])
        totT = gpool.tile([16, 8], FH, tag="totT")
        nc.vector.tensor_copy(out=totT[:], in_=totT_ps[:])
        pref_ps = psA.tile([8, NT], FP, space="PSUM", tag="small", bufs=2)
        nc.tensor.matmul(out=pref_ps[:], lhsT=totT[:], rhs=tri16[:],
                         start=True, stop=True)
        pref = gpool.tile([8, NT], FP, tag="pref_sb")
        nc.vector.tensor_copy(out=pref[:], in_=pref_ps[:])
        for i in range(NT):
            nc.vector.tensor_scalar_add(
                p_t[:, i * 128:(i + 1) * 128],
                p_t[:, i * 128:(i + 1) * 128],
                pref[:, i:i + 1],
            )

        # pm = (P+1)*M - 1 per token tile; also write cw (hi, lo) bf16 pairs
        # into the spare columns of the gather source
        pm_tiles = []
        for i in range(NT):
            pl_ps = psA.tile([128, 8], FP, space="PSUM", tag="small", bufs=2)
            nc.tensor.transpose(out=pl_ps[:], in_=p_t[:, i * 128:(i + 1) * 128],
                                identity=ident[:8, :8])
            mlf = gpool.tile([128, EL], FH, tag="mlf")
            nc.vector.tensor_scalar(
                out=mlf[:], in0=cwl_tiles[i][:], scalar1=0.0, scalar2=None,
                op0=mybir.AluOpType.is_gt,
            )
            pm = ppool.tile([128, EL], FH, tag="pm", bufs=16)
            nc.vector.tensor_scalar_add(pm[:], pl_ps[:], 1.0)
            nc.vector.tensor_mul(out=pm[:], in0=pm[:], in1=mlf[:])
            nc.vector.tensor_scalar(
                out=pm[:], in0=pm[:], scalar1=1.0, scalar2=None,
                op0=mybir.AluOpType.subtract,
            )
            pm_tiles.append(pm)
        # cw bf16 (hi, lo) pairs -> x_aug[:, D:] (needed only before the
        # x-gathers, so emitted after the pm chain to unblock expert 0)
        for i in range(NT):
            wpair = gpool.tile([128, EL, 2], BF, tag="wpair", bufs=2)
            nc.vector.tensor_copy(out=wpair[:, :, 0], in_=cwl_tiles[i][:])
            wres = gpool.tile([128, EL], FP, tag="wres", bufs=2)
            nc.vector.tensor_tensor(
                out=wres[:], in0=cwl_tiles[i][:], in1=wpair[:, :, 0],
                op=mybir.AluOpType.subtract,
            )
            nc.vector.tensor_copy(out=wpair[:, :, 1], in_=wres[:])
            nc.sync.dma_start(
                out=x_aug[i * 128:(i + 1) * 128, D:DA],
                in_=wpair[:],
            )

        # ---------------- phase F: per-expert g-build + FFN ----------------
        g_int = ppool.tile([128, NCH], I32, tag="gint")
        wcol = ppool.tile([128, NCH], FP, tag="wcol")

        prev_scatters = list(memset_insts)
        for e in range(EL):
            # slot -> token map for this expert: one-hot matmuls over 16 tiles
            gacc = psG.tile([3, C], FP, space="PSUM", tag="gacc", bufs=1)
            g_ps = gacc[:]
            for i in range(NT):
                q = gpool.tile([128, C], BF, tag="q", bufs=4)
                nc.vector.tensor_tensor(
                    out=q[:],
                    in0=pm_tiles[i][:, e:e + 1].to_broadcast([128, C]),
                    in1=iota384[:],
                    op=mybir.AluOpType.is_equal,
                )
                nc.tensor.matmul(
                    out=g_ps,
                    lhsT=tok3[:, 3 * i:3 * i + 3],
                    rhs=q[:],
                    start=(i == 0),
                    stop=(i == NT - 1),
                )
            gsb = gpool.tile([3, C], FP, tag="gsb", bufs=2)
            nc.vector.tensor_copy(out=gsb[:], in_=g_ps)
            for i in range(CCH):
                s = e * CCH + i
                gt_ps = psA.tile([128, 3], FP, space="PSUM", tag="small", bufs=2)
                nc.tensor.transpose(out=gt_ps[:], in_=gsb[:, i * 128:(i + 1) * 128],
                                    identity=ident[:3, :3])
                gt_sb = gpool.tile([128, 3], FP, tag="gt_sb")
                nc.vector.tensor_copy(out=gt_sb[:], in_=gt_ps[:])
                # gf = hi + lo + OOB*(1 - occ)  (pad slots -> OOB, skipped)
                gf = gpool.tile([128, 1], FP, tag="gf")
                nc.vector.tensor_scalar(
                    out=gf[:], in0=gt_sb[:, 2:3], scalar1=float(-OOB),
                    scalar2=float(OOB),
                    op0=mybir.AluOpType.mult, op1=mybir.AluOpType.add,
                )
                nc.vector.tensor_add(out=gf[:], in0=gf[:], in1=gt_sb[:, 0:1])
                nc.vector.tensor_add(out=gf[:], in0=gf[:], in1=gt_sb[:, 1:2])
                nc.vector.tensor_copy(out=g_int[:, s:s + 1], in_=gf[:])

            # weights for this expert (fp8 DoubleRow pair layout)
            wu_sb = []
            for a in range(D // 256):
                wtile = wpool.tile([128, 2, F], F8, tag="wu", bufs=16)
                nc.sync.dma_start(out=wtile[:], in_=wu8[e, a])
                wu_sb.append(wtile)
            wd_sb = []
            for b in range(F // 256):
                wtile = wpool.tile([128, 2, D], F8, tag="wd", bufs=8)
                nc.sync.dma_start(out=wtile[:], in_=wd8[e, b])
                wd_sb.append(wtile)

            # gather x rows (+ cw pair columns) for the 3 slot chunks
            xg_t = []
            for i in range(CCH):
                s = e * CCH + i
                xg = fpool.tile([128, DA], BF, tag="xg", bufs=4)
                nc.gpsimd.indirect_dma_start(
                    out=xg[:],
                    out_offset=None,
                    in_=x_aug[:, :],
                    in_offset=bass.IndirectOffsetOnAxis(ap=g_int[:, s:s + 1], axis=0),
                    bounds_check=T - 1,
                    oob_is_err=False,
                )
                xg_t.append(xg)
                nc.vector.tensor_add(out=wcol[:, s:s + 1],
                                     in0=xg[:, D + 2 * e:D + 2 * e + 1],
                                     in1=xg[:, D + 2 * e + 1:D + 2 * e + 2])

            # transpose x -> fp8 pair tiles [128(d), 2, C]
            xpair = []
            for a in range(D // 256):
                xp = fpool.tile([128, 2, C], F8, tag="xgT", bufs=8)
                for j in range(2):
                    kk = 2 * a + j
                    tr_ps = psA.tile([128, C], BF, space="PSUM", tag="trps", bufs=1)
                    for i in range(CCH):
                        nc.tensor.transpose(
                            out=tr_ps[:, i * 128:(i + 1) * 128],
                            in_=xg_t[i][:, kk * 128:(kk + 1) * 128],
                            identity=identb[:],
                        )
                    nc.vector.tensor_copy(out=xp[:, j, :], in_=tr_ps[:])
                xpair.append(xp)

            # up (fp8 DoubleRow): hT[f, c] = Wu^T x^T, silu -> fp8 pairs
            hpair = [fpool.tile([128, 2, C], F8, tag="hT", bufs=4,
                                name=f"hpair{b}")
                     for b in range(F // 256)]
            for ft in range(F // 128):
                h_ps = psA.tile([128, C], FP, space="PSUM", tag="hps", bufs=2)
                for a in range(D // 256):
                    nc.tensor.matmul(
                        out=h_ps[:],
                        lhsT=wu_sb[a][:, :, ft * 128:(ft + 1) * 128],
                        rhs=xpair[a][:],
                        perf_mode=DR,
                        start=(a == 0),
                        stop=(a == D // 256 - 1),
                    )
                sg = fpool.tile([128, C], FP, tag="sg", bufs=2)
                nc.scalar.activation(out=sg[:], in_=h_ps[:],
                                     func=mybir.ActivationFunctionType.Sigmoid)
                nc.vector.tensor_mul(out=hpair[ft // 2][:, ft % 2, :],
                                     in0=sg[:], in1=h_ps[:])

            # down (fp8 DoubleRow) per slot chunk, scale by wcol, scatter-add
            cur_scatters = []
            for i in range(CCH):
                s = e * CCH + i
                y_sb = fpool.tile([128, D], BF, tag="ysb", bufs=3)
                for nn in range(D // 512):
                    y_ps = psA.tile([128, 512], FP, space="PSUM", tag="yps", bufs=2)
                    for b in range(F // 256):
                        nc.tensor.matmul(
                            out=y_ps[:],
                            lhsT=hpair[b][:, :, i * 128:(i + 1) * 128],
                            rhs=wd_sb[b][:, :, nn * 512:(nn + 1) * 512],
                            perf_mode=DR,
                            start=(b == 0),
                            stop=(b == F // 256 - 1),
                        )
                    nc.scalar.activation(
                        out=y_sb[:, nn * 512:(nn + 1) * 512], in_=y_ps[:],
                        func=mybir.ActivationFunctionType.Copy,
                        scale=wcol[:, s:s + 1],
                    )
                sc = nc.gpsimd.indirect_dma_start(
                    out=acc[:, :],
                    out_offset=bass.IndirectOffsetOnAxis(ap=g_int[:, s:s + 1], axis=0),
                    in_=y_sb[:],
                    in_offset=None,
                    bounds_check=T - 1,
                    oob_is_err=False,
                    compute_op=mybir.AluOpType.add,
                )
                for p in prev_scatters:
                    add_dep_helper(sc.ins, p)
                cur_scatters.append(sc.ins)
            prev_scatters = cur_scatters

        # ---------------- ReduceScatter ----------------
        if NO_RS:
            rs = nc.sync.dma_start(out=rs_out[:, :], in_=acc[0:TS, :])
        else:
            rs = nc.gpsimd.collective_compute(
                "ReduceScatter",
                mybir.AluOpType.add,
                ins=[acc.ap().opt()],
                outs=[rs_out.ap().opt()],
                replica_groups=[list(range(N_CORES))],
            )
        for p in prev_scatters:
            add_dep_helper(rs.ins, p)

        # ---------------- shared expert on the token shard (overlaps RS) ----------------
        wsu_sb = []
        for kk in range(D // 128):
            wtile = wpool.tile([128, F], BF, tag="wsu", bufs=8)
            nc.sync.dma_start(out=wtile[:], in_=wsu[kk * 128:(kk + 1) * 128, :])
            wsu_sb.append(wtile)
        wsd_sb = []
        for kk in range(F // 128):
            wtile = wpool.tile([128, D], BF, tag="wsd", bufs=4)
            nc.sync.dma_start(out=wtile[:], in_=wsd[kk * 128:(kk + 1) * 128, :])
            wsd_sb.append(wtile)
        xts_r = []
        for kk in range(D // 128):
            xr = fpool.tile([128, TS], BF, tag="xr", bufs=8)
            nc.vector.tensor_copy(out=xr[:], in_=xts_sb[kk][:])
            xts_r.append(xr)
        hsT = []
        for ft in range(F // 128):
            h_ps = psA.tile([128, TS], FP, space="PSUM", tag="hps", bufs=2)
            for kk in range(D // 128):
                nc.tensor.matmul(
                    out=h_ps[:],
                    lhsT=wsu_sb[kk][:, ft * 128:(ft + 1) * 128],
                    rhs=xts_r[kk][:],
                    start=(kk == 0),
                    stop=(kk == D // 128 - 1),
                )
            h_sb = fpool.tile([128, TS], BF, tag="hsT", bufs=4)
            sg = fpool.tile([128, TS], FP, tag="sg", bufs=2)
            nc.scalar.activation(out=sg[:], in_=h_ps[:],
                                 func=mybir.ActivationFunctionType.Sigmoid)
            nc.vector.tensor_mul(out=h_sb[:], in0=sg[:], in1=h_ps[:])
            hsT.append(h_sb)
        ys_tiles = []
        for ttile in range(TS // 128):
            ys_sb = fpool.tile([128, D], FP, tag="yssb", bufs=2)
            for nn in range(D // 512):
                y_ps = psA.tile([128, 512], FP, space="PSUM", tag="yps", bufs=2)
                for kk in range(F // 128):
                    nc.tensor.matmul(
                        out=y_ps[:],
                        lhsT=hsT[kk][:, ttile * 128:(ttile + 1) * 128],
                        rhs=wsd_sb[kk][:, nn * 512:(nn + 1) * 512],
                        start=(kk == 0),
                        stop=(kk == F // 128 - 1),
                    )
                nc.any.tensor_copy(out=ys_sb[:, nn * 512:(nn + 1) * 512], in_=y_ps[:])
            ys_tiles.append(ys_sb)

        # ---------------- final: out_shard = rs_out + shared ----------------
        for ttile in range(TS // 128):
            rt = fpool.tile([128, D], BF, tag="rt", bufs=2)
            ld = nc.sync.dma_start(out=rt[:], in_=rs_out[ttile * 128:(ttile + 1) * 128, :])
            add_dep_helper(ld.ins, rs.ins)
            ot = fpool.tile([128, D], FP, tag="ot", bufs=2)
            nc.vector.tensor_add(out=ot[:], in0=rt[:], in1=ys_tiles[ttile][:])
            nc.sync.dma_start(out=out_shard[ttile * 128:(ttile + 1) * 128, :], in_=ot[:])

    return nc


_CACHED = {}


def _get_compiled():
    if "nc" not in _CACHED:
        nc = build_kernel()
        nc.compile()
        _CACHED["nc"] = nc
    return _CACHED["nc"]


def make_in_maps(x, centroids, expert_biases, Ws_up, Ws_down, W_up, W_down):
    bf_np = mybir.dt.np(BF)
    f8_np = mybir.dt.np(F8)
    xf = np.ascontiguousarray(np.asarray(x, dtype=np.float32).reshape(T, D))
    x_aug = np.zeros((T, DA), dtype=bf_np)
    x_aug[:, :D] = xf.astype(bf_np)
    cenT = np.ascontiguousarray(np.asarray(centroids, dtype=np.float32).T)
    bias = np.tile(np.asarray(expert_biases, dtype=np.float32)[None, :], (128, 1))
    bias = np.ascontiguousarray(bias)
    wsu_h = np.ascontiguousarray(np.asarray(Ws_up, dtype=np.float32)).astype(bf_np)
    wsd_h = np.ascontiguousarray(np.asarray(Ws_down, dtype=np.float32)).astype(bf_np)
    # fp8 DoubleRow pair layout: [E, K/256, 128, 2, N]
    wu_bf = np.asarray(W_up, dtype=np.float32).astype(bf_np).astype(np.float32)
    wd_bf = np.asarray(W_down, dtype=np.float32).astype(bf_np).astype(np.float32)
    wu_p = wu_bf.reshape(E, D // 256, 2, 128, F).transpose(0, 1, 3, 2, 4)
    wd_p = wd_bf.reshape(E, F // 256, 2, 128, D).transpose(0, 1, 3, 2, 4)
    wu_p = np.ascontiguousarray(wu_p).astype(f8_np)
    wd_p = np.ascontiguousarray(wd_p).astype(f8_np)
    ident_np, ucomb_np, tri16_np, iota_np, tok3_np = _host_constants()
    consts = {
        "ident_c": ident_np,
        "ucomb_c": ucomb_np.astype(bf_np),
        "tri16_c": tri16_np.astype(mybir.dt.np(FH)),
        "iota_c": iota_np.astype(mybir.dt.np(FH)),
        "tok3_c": tok3_np.astype(bf_np),
    }
    in_maps = []
    for c in range(N_CORES):
        sel = np.zeros((E, EL), dtype=np.float32)
        for j in range(EL):
            sel[c * EL + j, j] = 1.0
        in_maps.append({
            **consts,
            "sel64": sel,
            "xTs": np.ascontiguousarray(xf[c * TS:(c + 1) * TS].T),
            "cenT": cenT,
            "bias128": bias,
            "x_aug": x_aug,
            "wu8": wu_p[c * EL:(c + 1) * EL],
            "wd8": wd_p[c * EL:(c + 1) * EL],
            "wsu": wsu_h,
            "wsd": wsd_h,
        })
    return in_maps


def kernel(x, centroids, expert_biases, Ws_up, Ws_down, W_up, W_down,
           _trace=False):
    from concourse.bass_utils import run_bass_kernel_spmd

    nc = _get_compiled()
    in_maps = make_in_maps(x, centroids, expert_biases, Ws_up, Ws_down,
                           W_up, W_down)
    r = run_bass_kernel_spmd(nc, in_maps, core_ids=list(range(N_CORES)),
                             trace=_trace)
    shards = [r.results[c]["out_shard"] for c in range(N_CORES)]
    out = np.concatenate(shards, axis=0).reshape(B, S, D).astype(np.float32)
    if _trace:
        _CACHED["last_result"] = r
    return out
